# revision 16
# baseline (speedup 1.0000x reference)
"""BrainRNN Trainium2 kernel: 8-core tensor-parallel Bass/Tile implementation.

v2 design (collective-latency-aware):

The per-layer serial chain in the v1 kernel (sigmoid -> AllGather of the
layer activation -> next layer's hidden matmuls) cost ~9.3us/layer = ~75us
of the 128us runtime.  v2 restructures so no collective sits on the layer
chain:

  * Terms depending only on the (constant) hidden state h — the recurrent
    masked matmuls — and on >=3-layer-old activations — the stale skip
    blocks — are row-sharded across the 8 cores exactly as the hint
    suggests.  Their per-core partial sums (128 rows, bias folded in) are
    AllGathered per layer, but each gather has >=3 chain-steps of slack, so
    its ~10us path latency (cast -> DMA -> AG -> reload) hides completely.
  * Terms needing fresh activations — the hidden-layer matmul (needs
    xx[l-1]) and the freshest skip diagonal block (needs xx[l-2]) — are
    computed REPLICATED on every core (full 1024 rows).  Every core then
    forms the full preactivation (replicated psum + gathered partials via
    identity-matmul accumulate) and the full sigmoid, so xx[l] is fully
    resident everywhere and the chain is just ACT -> PE -> ACT per layer
    (~2-4us).

All masked weights are staged f8e4m3 (x64 scale, exact-product masks) and
kept SBUF-RESIDENT (~147KB/partition): a one-time prep phase streams W and
adjacency chunks on the two HWDGE rings and applies masks on DVE/GpSimd,
writing straight into the resident slabs.  The steady-state body does no
weight DMA at all.  Sigmoids un-scale via ACT's scale argument; the f8
quantization keeps the end-to-end rel err ~8e-3 (gate 2e-2).
"""

import sys

sys.path.insert(0, "/opt/trn_rl_repo")

import numpy as np

D = 1024
L = 8
N = 8192
B = 32
P = 128
NC = 8
S = 64.0  # f8 weight pre-scale (power of 2; undone in the sigmoid)

_CACHE = {}

PREMASK = False  # True: host applies adjacency masks (device prep = plain DMA)
CHUNK = 2048  # prep streaming chunk, in slab columns
TIMING_BUILD_KW = {}


def _build(spmd=True, reps=1, ag=True, premask=None, chain_only=False,
           dma_only=False, debug_xx=False):
    if premask is None:
        premask = PREMASK
    import concourse.bacc as bacc
    import concourse.tile as tile
    import concourse.mybir as mybir

    F32 = mybir.dt.float32
    F16 = mybir.dt.float16
    F8 = mybir.dt.float8e4
    CPY = mybir.ActivationFunctionType.Copy
    SIG = mybir.ActivationFunctionType.Sigmoid

    nc = bacc.Bacc(
        "TRN2", target_bir_lowering=False, debug=False, num_devices=NC if spmd else 1
    )

    # ---- DRAM I/O ------------------------------------------------------
    ht_d = nc.dram_tensor("ht", [P, 64 * B], F16, kind="ExternalInput")
    xt_d = nc.dram_tensor("xt", [P, 2 * B], F16, kind="ExternalInput")
    winT_d = nc.dram_tensor("winT", [P, 2 * D], F16, kind="ExternalInput")
    eye_d = nc.dram_tensor("eye", [P, P], F16, kind="ExternalInput")
    bin_d = nc.dram_tensor("bin", [P, 1], F32, kind="ExternalInput")
    bh_d = nc.dram_tensor("bh", [P, L - 1], F32, kind="ExternalInput")
    woT_d = nc.dram_tensor("woT", [P, 8 * 64], F16, kind="ExternalInput")
    bo_d = nc.dram_tensor("bo", [64, 1], F32, kind="ExternalInput")

    # sharded recurrent slabs: wr{k} [P, (7-k)*D] f8 (+ masks)
    wr_d = [nc.dram_tensor(f"wr{k}", [P, (7 - k) * D], F8, kind="ExternalInput")
            for k in range(7)]
    # sharded stale-skip slabs per layer l=3..7: [P, (l-2)*D]
    wss_d = {l: nc.dram_tensor(f"wss{l}", [P, (l - 2) * D], F8, kind="ExternalInput")
             for l in range(3, 8)}
    # replicated fresh-skip diagonal per layer l=2..7: [P, 8*D]
    wsd_d = {l: nc.dram_tensor(f"wsd{l}", [P, 8 * D], F8, kind="ExternalInput")
             for l in range(2, 8)}
    # replicated hidden slabs i=0..6: [P, 8*D]
    wh_d = [nc.dram_tensor(f"wh{i}", [P, 8 * D], F8, kind="ExternalInput")
            for i in range(7)]
    if not premask:
        ar_d = [nc.dram_tensor(f"ar{k}", [P, (7 - k) * D], F8, kind="ExternalInput")
                for k in range(7)]
        ass_d = {l: nc.dram_tensor(f"ass{l}", [P, (l - 2) * D], F8,
                                   kind="ExternalInput") for l in range(3, 8)}
        asd_d = {l: nc.dram_tensor(f"asd{l}", [P, 8 * D], F8, kind="ExternalInput")
                 for l in range(2, 8)}
        ah_d = [nc.dram_tensor(f"ah{i}", [P, 8 * D], F8, kind="ExternalInput")
                for i in range(7)]
    outT_d = nc.dram_tensor("outT", [64, B], F32, kind="ExternalOutput")
    dbg_d = [nc.dram_tensor(f"dbg{l}", [P, 8 * B], mybir.dt.float16,
                            kind="ExternalOutput") for l in range(L)] if debug_xx else None
    dbgp_d = [nc.dram_tensor(f"dbgp{l}", [P, 8 * B], mybir.dt.float16,
                             kind="ExternalOutput") for l in range(L)] if debug_xx else None
    dbgr_d = [nc.dram_tensor(f"dbgr{l}", [P, 8 * B], F32,
                             kind="ExternalOutput") for l in range(L)] if debug_xx else None

    with tile.TileContext(nc) as tc:
        with (
            tc.tile_pool(name="cst", bufs=1) as cst,
            tc.tile_pool(name="wbp", bufs=2) as wbp,
            tc.tile_pool(name="abp", bufs=2) as abp,
            tc.tile_pool(name="xxp", bufs=2) as xxp,
            tc.tile_pool(name="gp", bufs=2) as gp,
            tc.tile_pool(name="xsp", bufs=4) as xsp,
            tc.tile_pool(name="pss", bufs=4, space="PSUM") as pss,
            tc.tile_pool(name="pso", bufs=1, space="PSUM") as pso,
            tc.tile_pool(name="psr", bufs=3, space="PSUM") as psr,
            tc.tile_pool(name="dram", bufs=1, space="DRAM") as dram,
        ):
            # ---- resident constants ------------------------------------
            def cdma(name, shape, dt, src):
                t = cst.tile(shape, dt, tag=name, name=name)
                nc.sync.dma_start(t[:], src[:, :])
                return t

            ht_sb = cdma("ht", [P, 64 * B], F16, ht_d)
            xt_sb = cdma("xt", [P, 2 * B], F16, xt_d)
            winT_sb = cdma("winT", [P, 2 * D], F16, winT_d)
            eye_sb = cdma("eye", [P, P], F16, eye_d)
            bin_sb = cdma("bin", [P, 1], F32, bin_d)
            bh_sb = cdma("bh", [P, L - 1], F32, bh_d)
            woT_sb = cdma("woT", [P, 8 * 64], F16, woT_d)
            bo_sb = cst.tile([64, 1], F32, tag="bo")
            nc.sync.dma_start(bo_sb[:], bo_d[:, :])

            # ---- resident masked-weight slabs --------------------------
            rec_sb = [cst.tile([P, (7 - k) * D], F8, tag=f"rec{k}", name=f"rec{k}")
                      for k in range(7)]
            ss_sb = {l: cst.tile([P, (l - 2) * D], F8, tag=f"ss{l}", name=f"ss{l}")
                     for l in range(3, 8)}
            sd_sb = {l: cst.tile([P, 8 * D], F8, tag=f"sd{l}", name=f"sd{l}")
                     for l in range(2, 8)}
            whm_sb = [cst.tile([P, 8 * D], F8, tag=f"whm{i}", name=f"whm{i}")
                      for i in range(7)]

            prep_cnt = [0]

            def prep_slab(dst, w_dram, a_dram, cols):
                for c0 in range(0, cols, CHUNK):
                    cw = min(CHUNK, cols - c0)
                    i = prep_cnt[0]
                    prep_cnt[0] += 1
                    w_eng = nc.sync if i % 2 == 0 else nc.scalar
                    a_eng = nc.scalar if i % 2 == 0 else nc.sync
                    if premask:
                        w_eng.dma_start(dst[:, c0 : c0 + cw],
                                        w_dram[:, c0 : c0 + cw])
                        continue
                    wb = wbp.tile([P, CHUNK], F8, tag="w", name="wb")
                    w_eng.dma_start(wb[:, :cw], w_dram[:, c0 : c0 + cw])
                    ab = abp.tile([P, CHUNK], F8, tag="a", name="ab")
                    a_eng.dma_start(ab[:, :cw], a_dram[:, c0 : c0 + cw])
                    m_eng = nc.vector if i % 2 == 0 else nc.gpsimd
                    m_eng.tensor_mul(dst[:, c0 : c0 + cw], wb[:, :cw], ab[:, :cw])

            for k in range(7):
                prep_slab(rec_sb[k], wr_d[k], None if premask else ar_d[k],
                          (7 - k) * D)
            for l in range(3, 8):
                prep_slab(ss_sb[l], wss_d[l], None if premask else ass_d[l],
                          (l - 2) * D)
            for l in range(2, 8):
                prep_slab(sd_sb[l], wsd_d[l], None if premask else asd_d[l], 8 * D)
            for i in range(7):
                prep_slab(whm_sb[i], wh_d[i], None if premask else ah_d[i], 8 * D)

            # ---- per-rep state -----------------------------------------
            pgat_carry = {}  # next-rep gathered partials (software pipelining)
            for _rep in range(reps):
                xxT = [None] * L  # full layer activation [P, 8*B] f16
                pgat = [None] * L  # gathered partial [P, 8*B] f16

                def ag_pipeline(l):
                    """sharded partial (rec + stale skip) -> cast(+bias) ->
                    cci -> AllGather -> reload [P, 8*B]."""
                    ps = pss.tile([P, B], F32, tag="ps", name="ps")
                    n_rec = (7 - l) * 8 if l <= 6 else 0
                    n_ss = (l - 3 + 1) * 8 if l >= 3 else 0
                    tot = n_rec + n_ss
                    n = 0
                    if dma_only:
                        tot = 1
                    else:
                        base = (l + 1) * 8
                        for t in range(n_rec):
                            nc.tensor.matmul(
                                ps[:, :],
                                rec_sb[l][:, t * P : (t + 1) * P],
                                ht_sb[:, (base + t) * B : (base + t + 1) * B],
                                start=(n == 0),
                                stop=(n == tot - 1),
                            )
                            n += 1
                        for j in range(l - 2):
                            for t in range(8):
                                nc.tensor.matmul(
                                    ps[:, :],
                                    ss_sb[l][:, (j * 8 + t) * P : (j * 8 + t + 1) * P],
                                    xxT[j][:, t * B : (t + 1) * B],
                                    start=(n == 0),
                                    stop=(n == tot - 1),
                                )
                                n += 1
                    if dma_only:
                        nc.tensor.matmul(ps[:, :], eye_sb[:, 0:P],
                                         xt_sb[:, 0:B], start=True, stop=True)
                    bias = bin_sb[:, 0:1] if l == 0 else bh_sb[:, l - 1 : l]
                    xs = xsp.tile([P, B], F16, tag="xs", name="xs")
                    nc.scalar.activation(
                        xs[:], ps[:, :], mybir.ActivationFunctionType.Identity,
                        bias=bias, scale=1.0,
                    )
                    cci = dram.tile([P, B], F16, tag=f"cci{l}", name=f"cci{l}")
                    cco = dram.tile([NC * P, B], F16, tag=f"cco{l}", name=f"cco{l}")
                    nc.sync.dma_start(cci[:], xs[:])
                    if spmd and ag:
                        nc.gpsimd.collective_compute(
                            "AllGather",
                            mybir.AluOpType.bypass,
                            replica_groups=[list(range(NC))],
                            ins=[cci[:].opt()],
                            outs=[cco[:].opt()],
                        )
                    else:
                        for c in range(NC):
                            nc.sync.dma_start(cco[c * P : (c + 1) * P, :], cci[:])
                    pgat[l] = gp.tile([P, 8 * B], F16, tag=f"pg{l}", name=f"pg{l}")
                    nc.sync.dma_start(
                        pgat[l][:].rearrange("p (t b) -> p t b", t=8),
                        cco[:].rearrange("(t p) b -> p t b", p=P),
                    )

                # gathers with no fresh-activation dependency: first rep
                # emits them here; later reps already emitted them during the
                # previous rep's tail (uniform 3-step AG lead time)
                for l in (0, 1, 2):
                    if l in pgat_carry:
                        pgat[l] = pgat_carry.pop(l)
                    else:
                        ag_pipeline(l)

                # ---- layer chain ----------------------------------------
                for l in range(L):
                    rp = psr.tile([P, 8 * B], F32, tag="rp", name="rp")
                    n = 0
                    if not chain_only:
                        if l == 0:
                            for t in range(2):
                                for o in range(8):
                                    nc.tensor.matmul(
                                        rp[:, o * B : (o + 1) * B],
                                        winT_sb[:, t * D + o * P : t * D + (o + 1) * P],
                                        xt_sb[:, t * B : (t + 1) * B],
                                        start=(t == 0 and o == 0),
                                        stop=False,
                                    )
                        else:
                            wt = whm_sb[l - 1]
                            for t in range(8):
                                for o in range(8):
                                    nc.tensor.matmul(
                                        rp[:, o * B : (o + 1) * B],
                                        wt[:, (t * 8 + o) * P : (t * 8 + o + 1) * P],
                                        xxT[l - 1][:, t * B : (t + 1) * B],
                                        start=(t == 0 and o == 0),
                                        stop=False,
                                    )
                        if l >= 2:
                            wt = sd_sb[l]
                            for t in range(8):
                                for o in range(8):
                                    nc.tensor.matmul(
                                        rp[:, o * B : (o + 1) * B],
                                        wt[:, (t * 8 + o) * P : (t * 8 + o + 1) * P],
                                        xxT[l - 2][:, t * B : (t + 1) * B],
                                        start=False,
                                        stop=False,
                                    )
                        started = True
                    else:
                        started = False
                    nc.tensor.matmul(
                        rp[:, :],
                        eye_sb[:, 0:P],
                        pgat[l][:, :],
                        start=(not started),
                        stop=True,
                    )
                    xxT[l] = xxp.tile([P, 8 * B], F16, tag=f"xxT{l}", name=f"xxT{l}")
                    nc.scalar.activation(xxT[l][:], rp[:], SIG, scale=1.0 / S)
                    if debug_xx:
                        nc.sync.dma_start(dbg_d[l][:, :], xxT[l][:])
                        nc.sync.dma_start(dbgp_d[l][:, :], pgat[l][:])
                        rcp = xxp.tile([P, 8 * B], F32, tag=f"rcp{l}", name=f"rcp{l}")
                        nc.scalar.activation(rcp[:], rp[:], CPY, scale=1.0)
                        nc.sync.dma_start(dbgr_d[l][:, :], rcp[:])
                    if l + 3 < L:
                        ag_pipeline(l + 3)
                    elif _rep + 1 < reps:
                        ag_pipeline(l - 5)
                        pgat_carry[l - 5] = pgat[l - 5]

                # ---- output layer ---------------------------------------
                ops = pso.tile([P, B], F32, tag="ops", name="ops")
                for t in range(8):
                    nc.tensor.matmul(
                        ops[:64, :],
                        woT_sb[:, t * 64 : (t + 1) * 64],
                        xxT[7][:, t * B : (t + 1) * B],
                        start=(t == 0),
                        stop=(t == 7),
                    )
                outT_sb = cst.tile([64, B], F32, tag="outT", name="outT_sb")
                nc.vector.tensor_scalar_add(outT_sb[:], ops[:64, :], bo_sb[:, 0:1])
                nc.sync.dma_start(outT_d[:, :], outT_sb[:])

    nc.compile()
    return nc


def _tilT(A, dtype):
    """natural W shard [d, n] -> lhsT slab [p, t*d], out[p, t*d+dd] = A[dd, t*128+p]."""
    d, n = A.shape
    T = n // P
    return np.ascontiguousarray(
        A.reshape(d, T, P).transpose(2, 1, 0).reshape(P, T * d).astype(dtype)
    )


def _tilM(M, dtype):
    """mask/activation slice [n, d] -> slab [p, t*d], out[p, t*d+dd] = M[t*128+p, dd]."""
    n, d = M.shape
    T = n // P
    return np.ascontiguousarray(
        M.reshape(T, P, d).transpose(1, 0, 2).reshape(P, T * d).astype(dtype)
    )


def _np_f8():
    import concourse.mybir as mybir

    return mybir.dt.np(mybir.dt.float8e4)


def _shard_inputs(inputs):
    F8 = _np_f8()
    x = np.asarray(inputs["x"], dtype=np.float32)
    h = np.asarray(inputs["hidden_states"], dtype=np.float32)
    adj = np.asarray(inputs["adj"])
    W_in = np.asarray(inputs["W_in"], dtype=np.float32)
    b_in = np.asarray(inputs["b_in"], dtype=np.float32)
    W_h = np.asarray(inputs["W_h"], dtype=np.float32)
    b_h = np.asarray(inputs["b_h"], dtype=np.float32)
    W_r = np.asarray(inputs["W_r"], dtype=np.float32)
    W_s = np.asarray(inputs["W_s"], dtype=np.float32)
    W_o = np.asarray(inputs["W_o"], dtype=np.float32)
    b_o = np.asarray(inputs["b_o"], dtype=np.float32)

    adjf = adj.astype(np.float32)

    # replicated pieces (identical on every core)
    ht = _tilM(h.T, np.float16)
    xt = _tilM(x.T, np.float16)
    winT = _tilT(S * W_in, np.float16)
    eye = np.eye(P, dtype=np.float16)
    woT = _tilT(W_o, np.float16)
    bo = np.ascontiguousarray(b_o).reshape(64, 1)

    def f8w(A):  # weight block -> x64 f8 slab
        return _tilT(S * A, F8)

    wh = [f8w(W_h[i]) for i in range(7)]
    ah = [_tilM(adjf[i * D : (i + 1) * D, (i + 1) * D : (i + 2) * D], F8)
          for i in range(7)]
    wsd = {l: f8w(W_s[l - 2][:, (l - 2) * D : (l - 1) * D]) for l in range(2, 8)}
    asd = {l: _tilM(adjf[(l - 2) * D : (l - 1) * D, l * D : (l + 1) * D], F8)
           for l in range(2, 8)}
    if PREMASK:
        whm = [f8w(W_h[i] * adjf[i * D : (i + 1) * D,
                                 (i + 1) * D : (i + 2) * D].T) for i in range(7)]
        wsdm = {l: f8w(W_s[l - 2][:, (l - 2) * D : (l - 1) * D]
                       * adjf[(l - 2) * D : (l - 1) * D, l * D : (l + 1) * D].T)
                for l in range(2, 8)}

    maps = []
    for c in range(NC):
        sl = slice(c * P, (c + 1) * P)
        m = {
            "ht": ht,
            "xt": xt,
            "winT": winT,
            "eye": eye,
            "bin": np.ascontiguousarray(S * b_in[sl]).reshape(P, 1),
            "bh": np.ascontiguousarray(S * b_h[:, sl].T),
            "woT": woT,
            "bo": bo,
        }
        for k in range(7):
            if PREMASK:
                mask = adjf[(k + 1) * D :, k * D + c * P : k * D + (c + 1) * P]
                m[f"wr{k}"] = f8w(W_r[k][sl, (k + 1) * D :] * mask.T)
            else:
                m[f"wr{k}"] = f8w(W_r[k][sl, (k + 1) * D :])
                m[f"ar{k}"] = _tilM(
                    adjf[(k + 1) * D :, k * D + c * P : k * D + (c + 1) * P], F8
                )
        for l in range(3, 8):
            j = l - 2
            if PREMASK:
                mask = adjf[: (l - 2) * D, l * D + c * P : l * D + (c + 1) * P]
                m[f"wss{l}"] = f8w(W_s[j][sl, : (l - 2) * D] * mask.T)
            else:
                m[f"wss{l}"] = f8w(W_s[j][sl, : (l - 2) * D])
                m[f"ass{l}"] = _tilM(
                    adjf[: (l - 2) * D, l * D + c * P : l * D + (c + 1) * P], F8
                )
        for l in range(2, 8):
            if PREMASK:
                m[f"wsd{l}"] = wsdm[l]
            else:
                m[f"wsd{l}"] = wsd[l]
                m[f"asd{l}"] = asd[l]
        for i in range(7):
            if PREMASK:
                m[f"wh{i}"] = whm[i]
            else:
                m[f"wh{i}"] = wh[i]
                m[f"ah{i}"] = ah[i]
        maps.append(m)
    return maps


def get_compiled():
    if "nc" not in _CACHE:
        _CACHE["nc"] = _build()
    return _CACHE["nc"]


def run(inputs, **run_kwargs):
    from concourse import bass_utils

    nc = get_compiled()
    in_maps = _shard_inputs(inputs)
    res = bass_utils.run_bass_kernel_spmd(
        nc, in_maps, core_ids=list(range(NC)), **run_kwargs
    )
    out = np.ascontiguousarray(res.results[0]["outT"].T.astype(np.float32))
    return out, res


def kernel(**inputs):
    out, _ = run(inputs)
    return out


# revision 24
# speedup vs baseline: 1.3583x; 1.3583x over previous
"""BrainRNN Trainium2 kernel: 8-core tensor-parallel Bass/Tile implementation.

v2 design (collective-latency-aware):

The per-layer serial chain in the v1 kernel (sigmoid -> AllGather of the
layer activation -> next layer's hidden matmuls) cost ~9.3us/layer = ~75us
of the 128us runtime.  v2 restructures so no collective sits on the layer
chain:

  * Terms depending only on the (constant) hidden state h — the recurrent
    masked matmuls — and on >=3-layer-old activations — the stale skip
    blocks — are row-sharded across the 8 cores exactly as the hint
    suggests.  Their per-core partial sums (128 rows, bias folded in) are
    AllGathered per layer, but each gather has >=3 chain-steps of slack, so
    its ~10us path latency (cast -> DMA -> AG -> reload) hides completely.
  * Terms needing fresh activations — the hidden-layer matmul (needs
    xx[l-1]) and the freshest skip diagonal block (needs xx[l-2]) — are
    computed REPLICATED on every core (full 1024 rows).  Every core then
    forms the full preactivation (replicated psum + gathered partials via
    identity-matmul accumulate) and the full sigmoid, so xx[l] is fully
    resident everywhere and the chain is just ACT -> PE -> ACT per layer
    (~2-4us).

All masked weights are staged f8e4m3 (x64 scale, exact-product masks) and
kept SBUF-RESIDENT (~147KB/partition): a one-time prep phase streams W and
adjacency chunks on the two HWDGE rings and applies masks on DVE/GpSimd,
writing straight into the resident slabs.  The steady-state body does no
weight DMA at all.  Sigmoids un-scale via ACT's scale argument; the f8
quantization keeps the end-to-end rel err ~8e-3 (gate 2e-2).
"""

import sys

sys.path.insert(0, "/opt/trn_rl_repo")

import numpy as np

D = 1024
L = 8
N = 8192
B = 32
P = 128
NC = 8
S = 64.0  # f8 weight pre-scale (power of 2; undone in the sigmoid)

_CACHE = {}

PREMASK = False  # True: host applies adjacency masks (device prep = plain DMA)
CHUNK = 2048  # prep streaming chunk, in slab columns
TIMING_BUILD_KW = {}


def _build(spmd=True, reps=1, ag=True, premask=None, chain_only=False,
           dma_only=False, debug_xx=False):
    if premask is None:
        premask = PREMASK
    import concourse.bacc as bacc
    import concourse.tile as tile
    import concourse.mybir as mybir

    F32 = mybir.dt.float32
    F16 = mybir.dt.float16
    F8 = mybir.dt.float8e4
    CPY = mybir.ActivationFunctionType.Copy
    SIG = mybir.ActivationFunctionType.Sigmoid

    nc = bacc.Bacc(
        "TRN2", target_bir_lowering=False, debug=False, num_devices=NC if spmd else 1
    )

    # ---- DRAM I/O ------------------------------------------------------
    ht_d = nc.dram_tensor("ht", [P, 64 * B], F16, kind="ExternalInput")
    xt_d = nc.dram_tensor("xt", [P, 2 * B], F16, kind="ExternalInput")
    winT_d = nc.dram_tensor("winT", [P, 2 * D], F16, kind="ExternalInput")
    eye_d = nc.dram_tensor("eye", [P, P], F16, kind="ExternalInput")
    bin_d = nc.dram_tensor("bin", [P, 1], F32, kind="ExternalInput")
    bh_d = nc.dram_tensor("bh", [P, L - 1], F32, kind="ExternalInput")
    woT_d = nc.dram_tensor("woT", [P, 8 * 64], F16, kind="ExternalInput")
    bo_d = nc.dram_tensor("bo", [64, 1], F32, kind="ExternalInput")

    # sharded recurrent slabs: wr{k} [P, (7-k)*D] f8 (+ masks)
    wr_d = [nc.dram_tensor(f"wr{k}", [P, (7 - k) * D], F8, kind="ExternalInput")
            for k in range(7)]
    # sharded stale-skip slabs per layer l=3..7: [P, (l-2)*D]
    wss_d = {l: nc.dram_tensor(f"wss{l}", [P, (l - 2) * D], F8, kind="ExternalInput")
             for l in range(4, 8)}
    # replicated fresh-skip diagonal per layer l=2..7: [P, 8*D]
    wsd_d = {l: nc.dram_tensor(f"wsd{l}", [P, 8 * D], F8, kind="ExternalInput")
             for l in range(2, 8)}
    wsd2_d = nc.dram_tensor("wsd23", [P, 8 * D], F8, kind="ExternalInput")
    # replicated hidden slabs i=0..6: [P, 8*D]
    wh_d = [nc.dram_tensor(f"wh{i}", [P, 8 * D], F8, kind="ExternalInput")
            for i in range(7)]
    if not premask:
        ar_d = [nc.dram_tensor(f"ar{k}", [P, (7 - k) * D], F8, kind="ExternalInput")
                for k in range(7)]
        ass_d = {l: nc.dram_tensor(f"ass{l}", [P, (l - 2) * D], F8,
                                   kind="ExternalInput") for l in range(4, 8)}
        asd_d = {l: nc.dram_tensor(f"asd{l}", [P, 8 * D], F8, kind="ExternalInput")
                 for l in range(2, 8)}
        asd2_d = nc.dram_tensor("asd23", [P, 8 * D], F8, kind="ExternalInput")
        ah_d = [nc.dram_tensor(f"ah{i}", [P, 8 * D], F8, kind="ExternalInput")
                for i in range(7)]
    outT_d = nc.dram_tensor("outT", [64, B], F32, kind="ExternalOutput")
    dbg_d = [nc.dram_tensor(f"dbg{l}", [P, 8 * B], mybir.dt.float16,
                            kind="ExternalOutput") for l in range(L)] if debug_xx else None
    dbgp_d = [nc.dram_tensor(f"dbgp{l}", [P, 8 * B], mybir.dt.float16,
                             kind="ExternalOutput") for l in range(L)] if debug_xx else None
    dbgr_d = [nc.dram_tensor(f"dbgr{l}", [P, 8 * B], F32,
                             kind="ExternalOutput") for l in range(L)] if debug_xx else None

    with tile.TileContext(nc) as tc:
        with (
            tc.tile_pool(name="cst", bufs=1) as cst,
            tc.tile_pool(name="wbp", bufs=2) as wbp,
            tc.tile_pool(name="abp", bufs=2) as abp,
            tc.tile_pool(name="xxp", bufs=2) as xxp,
            tc.tile_pool(name="gp", bufs=2) as gp,
            tc.tile_pool(name="xsp", bufs=4) as xsp,
            tc.tile_pool(name="pss", bufs=4, space="PSUM") as pss,
            tc.tile_pool(name="pso", bufs=1, space="PSUM") as pso,
            tc.tile_pool(name="psr", bufs=3, space="PSUM") as psr,
            tc.tile_pool(name="dram", bufs=1, space="DRAM") as dram,
        ):
            # ---- resident constants ------------------------------------
            def cdma(name, shape, dt, src):
                t = cst.tile(shape, dt, tag=name, name=name)
                nc.sync.dma_start(t[:], src[:, :])
                return t

            ht_sb = cdma("ht", [P, 64 * B], F16, ht_d)
            xt_sb = cdma("xt", [P, 2 * B], F16, xt_d)
            winT_sb = cdma("winT", [P, 2 * D], F16, winT_d)
            eye_sb = cdma("eye", [P, P], F16, eye_d)
            bin_sb = cdma("bin", [P, 1], F32, bin_d)
            bh_sb = cdma("bh", [P, L - 1], F32, bh_d)
            woT_sb = cdma("woT", [P, 8 * 64], F16, woT_d)
            bo_sb = cst.tile([64, 1], F32, tag="bo")
            nc.sync.dma_start(bo_sb[:], bo_d[:, :])

            # ---- resident masked-weight slabs --------------------------
            rec_sb = [cst.tile([P, (7 - k) * D], F8, tag=f"rec{k}", name=f"rec{k}")
                      for k in range(7)]
            ss_sb = {l: cst.tile([P, (l - 2) * D], F8, tag=f"ss{l}", name=f"ss{l}")
                     for l in range(4, 8)}
            sd_sb = {l: cst.tile([P, 8 * D], F8, tag=f"sd{l}", name=f"sd{l}")
                     for l in range(2, 8)}
            sd2_sb = cst.tile([P, 8 * D], F8, tag="sd23", name="sd23")
            whm_sb = [cst.tile([P, 8 * D], F8, tag=f"whm{i}", name=f"whm{i}")
                      for i in range(7)]

            prep_cnt = [0]

            def prep_slab(dst, w_dram, a_dram, cols):
                for c0 in range(0, cols, CHUNK):
                    cw = min(CHUNK, cols - c0)
                    i = prep_cnt[0]
                    prep_cnt[0] += 1
                    w_eng = nc.sync if i % 2 == 0 else nc.scalar
                    a_eng = nc.scalar if i % 2 == 0 else nc.sync
                    if premask:
                        w_eng.dma_start(dst[:, c0 : c0 + cw],
                                        w_dram[:, c0 : c0 + cw])
                        continue
                    wb = wbp.tile([P, CHUNK], F8, tag="w", name="wb")
                    w_eng.dma_start(wb[:, :cw], w_dram[:, c0 : c0 + cw])
                    ab = abp.tile([P, CHUNK], F8, tag="a", name="ab")
                    a_eng.dma_start(ab[:, :cw], a_dram[:, c0 : c0 + cw])
                    m_eng = nc.vector if i % 2 == 0 else nc.gpsimd
                    m_eng.tensor_mul(dst[:, c0 : c0 + cw], wb[:, :cw], ab[:, :cw])

            for k in range(7):
                prep_slab(rec_sb[k], wr_d[k], None if premask else ar_d[k],
                          (7 - k) * D)
            for l in range(4, 8):
                prep_slab(ss_sb[l], wss_d[l], None if premask else ass_d[l],
                          (l - 2) * D)
            prep_slab(sd2_sb, wsd2_d, None if premask else asd2_d, 8 * D)
            for l in range(2, 8):
                prep_slab(sd_sb[l], wsd_d[l], None if premask else asd_d[l], 8 * D)
            for i in range(7):
                prep_slab(whm_sb[i], wh_d[i], None if premask else ah_d[i], 8 * D)

            # ---- per-rep state -----------------------------------------
            # Collectives are expensive under concurrency (~7-11us effective
            # each), so the 8 per-layer partial gathers are BATCHED into 3
            # collectives per rep:
            #   batch A (at step 0): layer 3 partial + NEXT rep's layers
            #                        0,1,2 partials (those depend only on h)
            #   batch B (at step 2): layers 4,5
            #   batch C (at step 4): layers 6,7
            pgat_carry = {}  # next-rep gathered partials (software pipelining)
            for _rep in range(reps):
                xxT = [None] * L  # full layer activation [P, 8*B] f16
                pgat = [None] * L  # gathered partial [P, 8*B] f16

                def partial_cast(l, dst, col):
                    """sharded partial (rec + stale skip) for layer l ->
                    Identity cast (+bias, x S) into dst[:, col*B:(col+1)*B]."""
                    ps = pss.tile([P, B], F32, tag="ps", name="ps")
                    n_rec = (7 - l) * 8 if l <= 6 else 0
                    n_ss = (l - 2) * 8 if l >= 4 else 0
                    tot = n_rec + n_ss
                    n = 0
                    if dma_only:
                        nc.tensor.matmul(ps[:, :], eye_sb[:, 0:P],
                                         xt_sb[:, 0:B], start=True, stop=True)
                    else:
                        base = (l + 1) * 8
                        for t in range(n_rec):
                            nc.tensor.matmul(
                                ps[:, :],
                                rec_sb[l][:, t * P : (t + 1) * P],
                                ht_sb[:, (base + t) * B : (base + t + 1) * B],
                                start=(n == 0),
                                stop=(n == tot - 1),
                            )
                            n += 1
                        for j in range(l - 2 if l >= 4 else 0):
                            for t in range(8):
                                nc.tensor.matmul(
                                    ps[:, :],
                                    ss_sb[l][:, (j * 8 + t) * P : (j * 8 + t + 1) * P],
                                    xxT[j][:, t * B : (t + 1) * B],
                                    start=(n == 0),
                                    stop=(n == tot - 1),
                                )
                                n += 1
                    bias = bin_sb[:, 0:1] if l == 0 else bh_sb[:, l - 1 : l]
                    nc.scalar.activation(
                        dst[:, col * B : (col + 1) * B], ps[:, :],
                        mybir.ActivationFunctionType.Identity,
                        bias=bias, scale=1.0,
                    )

                def batch_ag(name, xs, nl, reloads):
                    """one collective for nl stacked partials; reloads is a
                    list of (col, sink) where sink(tile) receives the
                    gathered [P, 8*B] partial for that column."""
                    cci = dram.tile([P, nl * B], F16, tag=f"cci{name}",
                                    name=f"cci{name}")
                    cco = dram.tile([NC * P, nl * B], F16, tag=f"cco{name}",
                                    name=f"cco{name}")
                    nc.sync.dma_start(cci[:], xs[:, : nl * B])
                    if spmd and ag:
                        nc.gpsimd.collective_compute(
                            "AllGather",
                            mybir.AluOpType.bypass,
                            replica_groups=[list(range(NC))],
                            ins=[cci[:].opt()],
                            outs=[cco[:].opt()],
                        )
                    else:
                        for c in range(NC):
                            nc.sync.dma_start(cco[c * P : (c + 1) * P, :], cci[:])
                    for col, lab, sink in reloads:
                        g = gp.tile([P, 8 * B], F16, tag=f"pg{lab}", name=f"pg{lab}")
                        nc.scalar.dma_start(
                            g[:].rearrange("p (t b) -> p t b", t=8),
                            cco[:, col * B : (col + 1) * B].rearrange(
                                "(t p) b -> p t b", p=P
                            ),
                        )
                        sink(g)

                def sink_cur(l):
                    def f(g):
                        pgat[l] = g
                    return f

                def sink_carry(l):
                    def f(g):
                        pgat_carry[l] = g
                    return f

                if not pgat_carry:
                    # first rep: upfront gather of the h-only layers 0..3
                    xs = xsp.tile([P, 4 * B], F16, tag="xsA", name="xsA")
                    for col, l in enumerate((0, 1, 2, 3)):
                        partial_cast(l, xs, col)
                    batch_ag("A0", xs, 4,
                             [(c, f"i{l}", sink_cur(l)) for c, l in
                              enumerate((0, 1, 2, 3))])
                else:
                    for l in (0, 1, 2, 3):
                        pgat[l] = pgat_carry.pop(l)

                def post_step(l):
                    """gather-batch emission schedule, after sigmoid_l."""
                    if l == 2:
                        xs = xsp.tile([P, 2 * B], F16, tag="xsB", name="xsB")
                        partial_cast(4, xs, 0)
                        partial_cast(5, xs, 1)
                        batch_ag("B", xs, 2,
                                 [(0, "4", sink_cur(4)), (1, "5", sink_cur(5))])
                    elif l == 4:
                        xs = xsp.tile([P, 2 * B], F16, tag="xsC", name="xsC")
                        partial_cast(6, xs, 0)
                        partial_cast(7, xs, 1)
                        batch_ag("C", xs, 2,
                                 [(0, "6", sink_cur(6)), (1, "7", sink_cur(7))])
                    elif l == 5 and _rep + 1 < reps:
                        # next rep's h-only partials {0..3}: inputs are
                        # constant, so gather them with 2.5 steps of lead
                        xs = xsp.tile([P, 4 * B], F16, tag="xsD", name="xsD")
                        rel = []
                        for col, m in enumerate((0, 1, 2, 3)):
                            partial_cast(m, xs, col)
                            rel.append((col, f"c{m}", sink_carry(m)))
                        batch_ag("D", xs, 4, rel)

                # ---- layer chain ----------------------------------------
                for l in range(L):
                    rp = psr.tile([P, 8 * B], F32, tag="rp", name="rp")
                    n = 0
                    if not chain_only:
                        # fresh-skip diags first (oldest input first): the
                        # PE crunches them during sigmoid_{l-1}
                        sds = []
                        if l == 3:
                            sds.append((sd2_sb, 0))
                        if l >= 2:
                            sds.append((sd_sb[l], l - 2))
                        for si, (wt, jin) in enumerate(sds):
                            for t in range(8):
                                for o in range(8):
                                    nc.tensor.matmul(
                                        rp[:, o * B : (o + 1) * B],
                                        wt[:, (t * 8 + o) * P : (t * 8 + o + 1) * P],
                                        xxT[jin][:, t * B : (t + 1) * B],
                                        start=(si == 0 and t == 0 and o == 0),
                                        stop=False,
                                    )
                        if l == 0:
                            for t in range(2):
                                for o in range(8):
                                    nc.tensor.matmul(
                                        rp[:, o * B : (o + 1) * B],
                                        winT_sb[:, t * D + o * P : t * D + (o + 1) * P],
                                        xt_sb[:, t * B : (t + 1) * B],
                                        start=(t == 0 and o == 0),
                                        stop=False,
                                    )
                        else:
                            wt = whm_sb[l - 1]
                            for t in range(8):
                                for o in range(8):
                                    nc.tensor.matmul(
                                        rp[:, o * B : (o + 1) * B],
                                        wt[:, (t * 8 + o) * P : (t * 8 + o + 1) * P],
                                        xxT[l - 1][:, t * B : (t + 1) * B],
                                        start=(l == 1 and t == 0 and o == 0),
                                        stop=False,
                                    )
                        started = True
                    else:
                        started = False
                    for o in range(8):
                        nc.tensor.matmul(
                            rp[:, o * B : (o + 1) * B],
                            eye_sb[:, 0:P],
                            pgat[l][:, o * B : (o + 1) * B],
                            start=(not started and o == 0),
                            stop=(o == 7),
                        )
                    xxT[l] = xxp.tile([P, 8 * B], F16, tag=f"xxT{l}", name=f"xxT{l}")
                    nc.scalar.activation(xxT[l][:], rp[:], SIG, scale=1.0 / S)
                    if debug_xx:
                        nc.sync.dma_start(dbg_d[l][:, :], xxT[l][:])
                        nc.sync.dma_start(dbgp_d[l][:, :], pgat[l][:])
                        rcp = xxp.tile([P, 8 * B], F32, tag=f"rcp{l}", name=f"rcp{l}")
                        nc.scalar.activation(rcp[:], rp[:], CPY, scale=1.0)
                        nc.sync.dma_start(dbgr_d[l][:, :], rcp[:])
                    post_step(l)

                # ---- output layer ---------------------------------------
                ops = pso.tile([P, B], F32, tag="ops", name="ops")
                for t in range(8):
                    nc.tensor.matmul(
                        ops[:64, :],
                        woT_sb[:, t * 64 : (t + 1) * 64],
                        xxT[7][:, t * B : (t + 1) * B],
                        start=(t == 0),
                        stop=(t == 7),
                    )
                outT_sb = cst.tile([64, B], F32, tag="outT", name="outT_sb")
                nc.vector.tensor_scalar_add(outT_sb[:], ops[:64, :], bo_sb[:, 0:1])
                nc.sync.dma_start(outT_d[:, :], outT_sb[:])



    nc.compile()
    return nc


def _tilT(A, dtype):
    """natural W shard [d, n] -> lhsT slab [p, t*d], out[p, t*d+dd] = A[dd, t*128+p]."""
    d, n = A.shape
    T = n // P
    return np.ascontiguousarray(
        A.reshape(d, T, P).transpose(2, 1, 0).reshape(P, T * d).astype(dtype)
    )


def _tilM(M, dtype):
    """mask/activation slice [n, d] -> slab [p, t*d], out[p, t*d+dd] = M[t*128+p, dd]."""
    n, d = M.shape
    T = n // P
    return np.ascontiguousarray(
        M.reshape(T, P, d).transpose(1, 0, 2).reshape(P, T * d).astype(dtype)
    )


def _np_f8():
    import concourse.mybir as mybir

    return mybir.dt.np(mybir.dt.float8e4)


def _shard_inputs(inputs):
    F8 = _np_f8()
    x = np.asarray(inputs["x"], dtype=np.float32)
    h = np.asarray(inputs["hidden_states"], dtype=np.float32)
    adj = np.asarray(inputs["adj"])
    W_in = np.asarray(inputs["W_in"], dtype=np.float32)
    b_in = np.asarray(inputs["b_in"], dtype=np.float32)
    W_h = np.asarray(inputs["W_h"], dtype=np.float32)
    b_h = np.asarray(inputs["b_h"], dtype=np.float32)
    W_r = np.asarray(inputs["W_r"], dtype=np.float32)
    W_s = np.asarray(inputs["W_s"], dtype=np.float32)
    W_o = np.asarray(inputs["W_o"], dtype=np.float32)
    b_o = np.asarray(inputs["b_o"], dtype=np.float32)

    adjf = adj.astype(np.float32)

    # replicated pieces (identical on every core)
    ht = _tilM(h.T, np.float16)
    xt = _tilM(x.T, np.float16)
    winT = _tilT(S * W_in, np.float16)
    eye = np.eye(P, dtype=np.float16)
    woT = _tilT(W_o, np.float16)
    bo = np.ascontiguousarray(b_o).reshape(64, 1)

    def f8w(A):  # weight block -> x64 f8 slab
        return _tilT(S * A, F8)

    wh = [f8w(W_h[i]) for i in range(7)]
    ah = [_tilM(adjf[i * D : (i + 1) * D, (i + 1) * D : (i + 2) * D], F8)
          for i in range(7)]
    wsd = {l: f8w(W_s[l - 2][:, (l - 2) * D : (l - 1) * D]) for l in range(2, 8)}
    wsd2 = f8w(W_s[1][:, 0:D])
    asd2 = _tilM(adjf[0:D, 3 * D : 4 * D], F8)
    asd = {l: _tilM(adjf[(l - 2) * D : (l - 1) * D, l * D : (l + 1) * D], F8)
           for l in range(2, 8)}
    if PREMASK:
        whm = [f8w(W_h[i] * adjf[i * D : (i + 1) * D,
                                 (i + 1) * D : (i + 2) * D].T) for i in range(7)]
        wsdm = {l: f8w(W_s[l - 2][:, (l - 2) * D : (l - 1) * D]
                       * adjf[(l - 2) * D : (l - 1) * D, l * D : (l + 1) * D].T)
                for l in range(2, 8)}
        wsd2m = f8w(W_s[1][:, 0:D] * adjf[0:D, 3 * D : 4 * D].T)

    maps = []
    for c in range(NC):
        sl = slice(c * P, (c + 1) * P)
        m = {
            "ht": ht,
            "xt": xt,
            "winT": winT,
            "eye": eye,
            "bin": np.ascontiguousarray(S * b_in[sl]).reshape(P, 1),
            "bh": np.ascontiguousarray(S * b_h[:, sl].T),
            "woT": woT,
            "bo": bo,
        }
        for k in range(7):
            if PREMASK:
                mask = adjf[(k + 1) * D :, k * D + c * P : k * D + (c + 1) * P]
                m[f"wr{k}"] = f8w(W_r[k][sl, (k + 1) * D :] * mask.T)
            else:
                m[f"wr{k}"] = f8w(W_r[k][sl, (k + 1) * D :])
                m[f"ar{k}"] = _tilM(
                    adjf[(k + 1) * D :, k * D + c * P : k * D + (c + 1) * P], F8
                )
        for l in range(4, 8):
            j = l - 2
            if PREMASK:
                mask = adjf[: (l - 2) * D, l * D + c * P : l * D + (c + 1) * P]
                m[f"wss{l}"] = f8w(W_s[j][sl, : (l - 2) * D] * mask.T)
            else:
                m[f"wss{l}"] = f8w(W_s[j][sl, : (l - 2) * D])
                m[f"ass{l}"] = _tilM(
                    adjf[: (l - 2) * D, l * D + c * P : l * D + (c + 1) * P], F8
                )
        if PREMASK:
            m["wsd23"] = wsd2m
        else:
            m["wsd23"] = wsd2
            m["asd23"] = asd2
        for l in range(2, 8):
            if PREMASK:
                m[f"wsd{l}"] = wsdm[l]
            else:
                m[f"wsd{l}"] = wsd[l]
                m[f"asd{l}"] = asd[l]
        for i in range(7):
            if PREMASK:
                m[f"wh{i}"] = whm[i]
            else:
                m[f"wh{i}"] = wh[i]
                m[f"ah{i}"] = ah[i]
        maps.append(m)
    return maps


def get_compiled():
    if "nc" not in _CACHE:
        _CACHE["nc"] = _build()
    return _CACHE["nc"]


def run(inputs, **run_kwargs):
    from concourse import bass_utils

    nc = get_compiled()
    in_maps = _shard_inputs(inputs)
    res = bass_utils.run_bass_kernel_spmd(
        nc, in_maps, core_ids=list(range(NC)), **run_kwargs
    )
    out = np.ascontiguousarray(res.results[0]["outT"].T.astype(np.float32))
    return out, res


def kernel(**inputs):
    out, _ = run(inputs)
    return out
